# revision 22
# baseline (speedup 1.0000x reference)
"""Trainium2 Bass kernel for word2vec-style binary log loss (negative sampling).

loss = sum_n -logsig(h_n . E[pos_n]) + sum_n mean_k -logsig(-h_n . E[neg_nk])

Strategy: data-parallel over the batch N across 8 NeuronCores; all 43008
score dot-products per core are computed ON THE TENSOR ENGINE as fp8
all-pairs matmuls (fp8 moving at 1 col/cycle; elementwise engines cannot
come close: a DVE multiply alone would cost 37us/core).

Host prep ships, per core, only the gathered embedding rows (fp8e4m3,
d-major: [128 d, 43008 cols]) plus the hidden vectors as zero-padded
block-diagonal fp8 stationaries.  On device, for each tile of 128 samples:
4 groups of 32 samples write PSUM rows [32g, 32g+32) (PE tile_position col
offsets 0/32/64/96; DoubleRow would halve PE time but its ISA check only
allows tile position (0,0), so each group is TWO plain fp8 matmuls - one
per zero-padded 16-sample half-stationary - accumulating into the same
PSUM [128, 336] bank).  True scores sit at columns with c%16 == p%16: ACT
downcasts PSUM to bf16, the DVE runs the diagonal mask-multiply in its 2x
16-bit mode plus a grouped 16-wide reduce -> scores [128, 21] per tile
(the last two tiles skip the ACT hop: shortest drain chain).  softplus
uses the table-stable ln(1+exp(-|s|)) form (the HW Ln/Exp tables are
garbage outside a small range), weighted accumulates on DVE placed after
the extraction stream, and a [4,32]-packed partial returns per core; the
host sums 8*128 partials.

The g stream (5.5 MB fp8/core, ~17us) and the PE (43008 cols, ~18us) are
the twin rooflines: all DMAs are issued up-front on one HWDGE queue in
exact consumption order (h quarters interleaved with ramped g chunks) so
transfers run back-to-back at full bandwidth while the PE chases them;
dummy matmuls bridge the PE p-state ramp so real work starts at full
clock; extraction, softplus and accumulates hide underneath.
"""

import os
import sys

for _p in ("/root/.axon_site/_ro/trn_rl_repo", "/opt/trn_rl_repo"):
    if os.path.isdir(_p):
        if _p in sys.path:
            sys.path.remove(_p)
        sys.path.insert(0, _p)

import types

import numpy as np
import ml_dtypes

import concourse.bacc as bacc
import concourse.tile as tile
from concourse import mybir

# Problem constants (hardcoded per contest rules).
N, D, V, K = 16384, 128, 1000000, 20
NCORES = 8
P = 128
R = K + 1               # 21 regions: 1 pos + 20 neg
NS = N // NCORES        # 2048 samples per core
J = 16                  # samples per DoubleRow k-tile subgroup
W = 2 * J               # 32 samples per matmul group
G = 4                   # groups per tile (128 samples)
C = R * J               # 336 moving cols per group (2 pairs per col)
CG = 2 * C              # 672 g columns per group
NT = NS // P            # 16 tiles per core
NGRP = NT * G           # 64 groups per core
HALVES = (14, 2)        # softplus tail split (tiles)
ACT_TABLE = "natural_log_exp_and_others"

BF16 = mybir.dt.bfloat16
F8 = mybir.dt.float8e4
F32 = mybir.dt.float32

FP8 = ml_dtypes.float8_e4m3


def _patch_act_tables(nc):
    """Force the combined Exp+Ln activation table so the softplus chain
    never reloads tables mid-stream. Table IDs are positional in
    act_info.json, so other tables are emptied rather than removed."""
    from concourse.bacc import get_activation_tables
    import bass_rust as _bass_rust

    def insert_act_table_loads(self):
        has_activation = any(
            isinstance(i, mybir.InstActivation)
            for b in self.main_func.blocks
            for i in b.instructions
        )
        if not has_activation:
            return
        tables = [
            (name, fns if name == ACT_TABLE else set())
            for name, fns in get_activation_tables(self.m.arch).items()
        ]
        _bass_rust.insert_act_table_loads(self, tables)

    nc.insert_act_table_loads = types.MethodType(insert_act_table_loads, nc)


def build_bass():
    nc = bacc.Bacc("TRN2", target_bir_lowering=False)
    _patch_act_tables(nc)
    t_g = nc.dram_tensor("g", [P, NGRP * CG], F8, kind="ExternalInput")
    t_h = nc.dram_tensor("h", [P, NGRP * W * 2], F8, kind="ExternalInput")
    t_m = nc.dram_tensor("m", [P, C], BF16, kind="ExternalInput")
    t_out = nc.dram_tensor("out", [4, 32], F32, kind="ExternalOutput")

    with (
        tile.TileContext(nc) as tc,
        tc.tile_pool(name="c", bufs=1) as cpool,
        tc.psum_pool(name="pp", bufs=6) as ppool,
        tc.psum_pool(name="wp", bufs=1) as wpool,
        tc.tile_pool(name="sp", bufs=4) as spool,
    ):
        hT = cpool.tile([P, NGRP * W * 2], F8)
        wmask = cpool.tile([P, C], BF16)
        gsb = cpool.tile([P, NGRP * CG], F8)
        scores = cpool.tile([P, NT, R], F32)

        # One ordered queue; everything up-front. h ships in quarters
        # interleaved with ramped g chunks so the first matmul's operands
        # land as early as possible while the stream stays saturated.
        HQ = NGRP * W * 2 // 4
        g_chunks = (6, 8, 8, 8, 8, 8, 8, 6, 2, 1, 1)
        assert sum(g_chunks) == NGRP
        sched = [("h", 0), ("g", 0), ("m", 0), ("g", 1), ("h", 1),
                 ("g", 2), ("h", 2), ("g", 3), ("h", 3)]
        sched += [("g", i) for i in range(4, len(g_chunks))]
        g0 = 0
        for kind, i in sched:
            if kind == "h":
                nc.sync.dma_start(out=hT[:, i * HQ : (i + 1) * HQ],
                                  in_=t_h[:, i * HQ : (i + 1) * HQ])
            elif kind == "m":
                nc.sync.dma_start(out=wmask[:], in_=t_m[:])
            else:
                ngrp = g_chunks[i]
                nc.sync.dma_start(
                    out=gsb[:, g0 * CG : (g0 + ngrp) * CG],
                    in_=t_g[:, g0 * CG : (g0 + ngrp) * CG],
                )
                g0 += ngrp

        # PE p-state warmup: ~3us of dummy matmuls during the idle window
        # before h lands, so real matmuls start at full clock
        wsrc = cpool.tile([P, 300], F8)
        nc.vector.memset(wsrc[:], 0.0)
        wps = wpool.tile([P, 300], F32, tag="warm")
        # one tiny matmul at the cold clock reaches the mid p-state, then
        # mid-clock work ramps toward full speed; sized to finish right as
        # the first h/g chunks land (~3.6us)
        nc.tensor.matmul(out=wps[0:32, 0:32], lhsT=wsrc[:, 0:32],
                         rhs=wsrc[:, 0:32], start=True, stop=True)
        for _ in range(12):
            nc.tensor.matmul(out=wps[0:32, 0:300], lhsT=wsrc[:, 0:32],
                             rhs=wsrc[:, 0:300], start=True, stop=True)

        # trigger the single activation-table load early, during the stream
        preheat = cpool.tile([P, 1], F32)
        nc.vector.memset(preheat[:], 0.0)
        nc.scalar.activation(
            out=preheat[:], in_=preheat[:],
            func=mybir.ActivationFunctionType.Abs,
        )

        lnx = cpool.tile([P, NT, R], F32)
        tmp_pos = cpool.tile([P, NT], F32)
        acc_pos = cpool.tile([P, 2], F32)
        tmp_neg = cpool.tile([P, NT, R - 1], F32)
        acc_neg = cpool.tile([P, 2], F32)

        def tail(hi, t0, t1, chain_only=False, stt_only=False):
            """softplus + weighted accumulate for tiles [t0, t1).
            Stable form only: the HW Ln/Exp activation TABLES are garbage
            for large arguments, so keep them in range with
            lnterm = ln(1 + exp(-|s|)), argument always in [1, 2]:
              neg (r>=1): softplus(s)  = max(s,0) + lnterm
              pos (r=0):  softplus(-s) = lnterm - min(s,0)
            ACT computes lnterm; the DVE accumulates are placed late in the
            stream so extraction is never head-of-line blocked."""
            s = scores[:, t0:t1, :]
            l3 = lnx[:, t0:t1, :]
            lf = l3.rearrange("p t r -> p (t r)")
            if not stt_only:
                nc.scalar.activation(out=lf, in_=s.rearrange("p t r -> p (t r)"),
                                     func=mybir.ActivationFunctionType.Abs)
                nc.scalar.activation(out=lf, in_=lf,
                                     func=mybir.ActivationFunctionType.Exp,
                                     scale=-1.0)
                nc.scalar.activation(out=lf, in_=lf,
                                     func=mybir.ActivationFunctionType.Ln,
                                     bias=1.0)
            if chain_only:
                return
            # pos: out = min(s,0) - lnterm; its sum is the NEGATED pos term
            nc.vector.scalar_tensor_tensor(
                out=tmp_pos[:, t0:t1], in0=s[:, :, 0], scalar=0.0,
                in1=l3[:, :, 0],
                op0=mybir.AluOpType.min, op1=mybir.AluOpType.subtract,
                accum_out=acc_pos[:, hi : hi + 1],
            )
            nc.vector.scalar_tensor_tensor(
                out=tmp_neg[:, t0:t1, :], in0=s[:, :, 1:R], scalar=0.0,
                in1=l3[:, :, 1:R],
                op0=mybir.AluOpType.max, op1=mybir.AluOpType.add,
                accum_out=acc_neg[:, hi : hi + 1],
            )

        for t in range(NT):
            ps = ppool.tile([P, C], F32, tag="ps")
            for g in range(G):
                # DoubleRow is rejected by the ISA check at nonzero PSUM
                # column offsets, so emulate it: two plain fp8 matmuls (one
                # per zero-padded k-tile stationary) accumulating into the
                # same PSUM rows.
                base = (t * G + g) * W * 2
                gbase = (t * G + g) * CG
                for k in range(2):
                    nc.tensor.matmul(
                        out=ps[g * W : (g + 1) * W, :],
                        lhsT=hT[:, base + k * W : base + (k + 1) * W],
                        rhs=gsb[:, gbase + k * C : gbase + (k + 1) * C],
                        start=(k == 0), stop=(k == 1),
                        tile_position=(0, g * W),
                    )
            msk = spool.tile([P, C], BF16, tag="msk")
            with nc.allow_low_precision("bf16 masked scores; tol 2e-2"):
                if t < NT - 2:
                    # steady state: ACT downcasts PSUM->bf16 so the DVE
                    # mask-mul runs in its 2x 16-bit mode and extraction
                    # outpaces the PE (keeps the drain from accumulating)
                    spc = spool.tile([P, C], BF16, tag="spc")
                    nc.scalar.activation(
                        out=spc[:], in_=ps[:],
                        func=mybir.ActivationFunctionType.Copy,
                    )
                    nc.vector.tensor_mul(out=msk[:], in0=spc[:], in1=wmask[:])
                else:
                    # last tiles: shortest chain, straight from PSUM
                    nc.vector.tensor_mul(out=msk[:], in0=ps[:], in1=wmask[:])
                nc.vector.tensor_reduce(
                    out=scores[:, t, :],
                    in_=msk[:].rearrange("p (r j) -> p r j", j=J),
                    axis=mybir.AxisListType.X, op=mybir.AluOpType.add,
                )
            if t == HALVES[0] - 1:
                tail(0, 0, HALVES[0], chain_only=True)
        tail(0, 0, HALVES[0], stt_only=True)
        tail(1, HALVES[0], NT)

        partial = cpool.tile([P, 32], F32)
        nc.vector.memset(partial[:], 0.0)
        accn = cpool.tile([P, 1], F32)
        nc.vector.tensor_reduce(out=accn[:], in_=acc_neg[:],
                                axis=mybir.AxisListType.X,
                                op=mybir.AluOpType.add)
        accp = cpool.tile([P, 1], F32)
        nc.vector.tensor_reduce(out=accp[:], in_=acc_pos[:],
                                axis=mybir.AxisListType.X,
                                op=mybir.AluOpType.add)
        # acc_pos carries the NEGATED positive term: partial = accn/K - accp
        nc.vector.scalar_tensor_tensor(
            out=partial[:, 0:1], in0=accn[:], scalar=1.0 / K,
            in1=accp[:], op0=mybir.AluOpType.mult,
            op1=mybir.AluOpType.subtract,
        )
        # pack [128,1] partials into 4 partition rows via the 32x32 stream
        # transpose so the output DMA is 4 descriptors of 128 B
        partial_t = cpool.tile([P, 32], F32)
        nc.vector.transpose(out=partial_t[:], in_=partial[:])
        nc.sync.dma_start(
            out=t_out[:],
            in_=partial_t[:].rearrange("(q s) c -> q s c", s=32)[:, 0, :],
        )
    nc.compile()
    return nc


# col -> (m, r) map for the g layout: col = ((t*G+g)*2+k)*C + r*J + j,
# sample m = t*128 + g*32 + k*16 + j  (so scores land at partition t*128+p)
def _col_maps():
    cols = np.arange(NGRP * CG)
    grp = cols // CG
    within = cols % CG
    k = within // C
    r = (within % C) // J
    j = within % J
    t = grp // G
    g = grp % G
    m = t * P + g * W + k * J + j
    return m, r


_M_OF_COL, _R_OF_COL = _col_maps()


def _wmask():
    p = np.arange(P)[:, None]
    c = np.arange(C)[None, :]
    return ((c % J) == (p % J)).astype(ml_dtypes.bfloat16)


_WMASK = _wmask()


def make_in_maps(hidden_state, label_idxes, neg_idxes, out_embed_weight):
    hidden_state = np.asarray(hidden_state, np.float32)
    table = np.asarray(out_embed_weight, np.float32)
    label = np.asarray(label_idxes).astype(np.int64, copy=False)
    negs = np.asarray(neg_idxes).astype(np.int64, copy=False)
    in_maps = []
    for cidx in range(NCORES):
        s0, s1 = cidx * NS, (cidx + 1) * NS
        idx2 = np.concatenate([label[s0:s1][:, None], negs[s0:s1]], axis=1)
        flat = idx2[_M_OF_COL, _R_OF_COL]          # [43008]
        gath = table[flat]                         # [43008, 128] f32
        g8 = np.ascontiguousarray(gath.T).astype(FP8)   # [128, 43008]

        hc = hidden_state[s0:s1].astype(FP8)       # [2048, 128]
        # hT zero-padded block-diag: [128, NGRP, 2, 32]
        hT = np.zeros((P, NGRP, 2, W), dtype=FP8)
        hcT = np.ascontiguousarray(hc.T).reshape(P, NT, G, 2, J)
        hT[:, :, 0, 0:J] = hcT.reshape(P, NGRP, 2, J)[:, :, 0, :]
        hT[:, :, 1, J:W] = hcT.reshape(P, NGRP, 2, J)[:, :, 1, :]
        in_maps.append({
            "g": g8,
            "h": np.ascontiguousarray(hT.reshape(P, NGRP * W * 2)),
            "m": _WMASK,
        })
    return in_maps


_NC_CACHE = {}


def get_nc():
    if "nc" not in _NC_CACHE:
        _NC_CACHE["nc"] = build_bass()
    return _NC_CACHE["nc"]


def kernel(hidden_state, label_idxes, neg_idxes, out_embed_weight):
    from concourse.bass_utils import run_bass_kernel_spmd

    nc = get_nc()
    in_maps = make_in_maps(hidden_state, label_idxes, neg_idxes, out_embed_weight)
    res = run_bass_kernel_spmd(nc, in_maps, core_ids=list(range(NCORES)))
    total = 0.0
    for r in res.results:
        total += float(np.asarray(r["out"], np.float64).sum())
    return np.float32(total)
